# revision 10
# baseline (speedup 1.0000x reference)
"""Trainium2 Bass kernel for a 2-layer tanh RNN (batch_first) + Linear head.

Problem: X[8192, 512, 4] -> out[8192, 2048] with H=32 hidden units per layer.
Sharding: pure data parallelism over batch across 8 NeuronCores (1024 b/core).

Per-core design (ACT/tanh-bound):
  - State s_t = [h0_t; h1_{t-1}] per lane (the layer-1 update is skewed one
    step so both layers update from available inputs in ONE matmul round +
    ONE tanh). Two lanes of 256 batch share each [128, 256] state tile.
  - 2 independent "chains" (2 lanes x 256 batch each) skewed in time so the
    scalar engine (tanh) never idles; PE work runs in the tanh shadow.
  - Each chain-step: 3 full [128,128]x[128,256] matmuls into one PSUM bank:
      1. Wbig (block-structured W_hh0/W_ih1/W_hh1 for both lanes)  @ s_{t-1}
      2. Xvar_{t%16} (row-selector x-injection W_ih0)              @ XT tile
      3. Zvar_{t%16} (head W_ll with per-t output-row placement)   @ s_t
    then one activation(Tanh, bias) [128, 256] PSUM->SBUF for the new state.
  - The head accumulates 16 timesteps into one PSUM bank; every 16 steps it
    is copied (+b_ll) to SBUF, PE-transposed back to [batch, (t,o)] layout,
    copied to SBUF and DMA'd to HBM.
  - X is loaded up-front into SBUF ([128b, 2048] tiles) and PE-transposed
    into [4i x 16t rows, batch] staging tiles, spread across rounds so the
    transposes hide in the tanh shadow.
"""

import os
import sys
import numpy as np

for _p in ("/opt/trn_rl_repo",):
    if _p not in sys.path:
        sys.path.insert(0, _p)

B, T, I, H, O = 8192, 512, 4, 32, 4
NCORES = 8
BS = B // NCORES          # 1024 batch per core
NB = 256                  # batch columns per chain (2 lanes of 256 -> 512 b)
NCH = 2                   # chains per core
TCH = 16                  # timesteps per x staging chunk and per z PSUM bank
NBLK = 33                 # weight blocks: Wbig + 16 Xvar + 16 Zvar

_CACHE = {}


def _build(t_steps):
    import concourse.bass as bass  # noqa: F401
    import concourse.tile as tile
    from concourse import bacc, mybir
    from contextlib import ExitStack

    f32 = mybir.dt.float32
    AF = mybir.ActivationFunctionType

    nc = bacc.Bacc(
        "TRN2", target_bir_lowering=False, debug=False, num_devices=NCORES
    )

    X_d = nc.dram_tensor("Xs", [BS, T * I], f32, kind="ExternalInput").ap()
    W_d = nc.dram_tensor("Wstk", [128, 128 * NBLK], f32, kind="ExternalInput").ap()
    bias_d = nc.dram_tensor("bias128", [128, 1], f32, kind="ExternalInput").ap()
    zbias_d = nc.dram_tensor("zbias", [128, 1], f32, kind="ExternalInput").ap()
    bias0_d = nc.dram_tensor("bias0", [128, 1], f32, kind="ExternalInput").ap()
    id_d = nc.dram_tensor("ident", [128, 128], f32, kind="ExternalInput").ap()
    zer_d = nc.dram_tensor("zer", [128, NB], f32, kind="ExternalInput").ap()
    out_d = nc.dram_tensor("out", [BS, T * O], f32, kind="ExternalOutput").ap()

    with tile.TileContext(nc) as tc, ExitStack() as ctx:
        consts = ctx.enter_context(tc.tile_pool(name="consts", bufs=1))
        xpool = ctx.enter_context(tc.tile_pool(name="xdata", bufs=1))
        spool = ctx.enter_context(tc.tile_pool(name="state", bufs=2))
        xtpool = ctx.enter_context(tc.tile_pool(name="xstage", bufs=2))
        xbpool = ctx.enter_context(tc.tile_pool(name="xbtmp", bufs=2))
        zspool = ctx.enter_context(tc.tile_pool(name="zstage", bufs=2))
        otpool = ctx.enter_context(tc.tile_pool(name="otstage", bufs=4))
        pspool = ctx.enter_context(tc.tile_pool(name="ps", bufs=1, space="PSUM"))
        pzpool = ctx.enter_context(tc.tile_pool(name="pz", bufs=1, space="PSUM"))
        pxpool = ctx.enter_context(tc.tile_pool(name="px", bufs=1, space="PSUM"))
        ptpool = ctx.enter_context(tc.tile_pool(name="pt", bufs=2, space="PSUM"))

        # --- constants / weights ---
        Wsb = consts.tile([128, 128 * NBLK], f32, tag="wstk", name="wstk")
        bias_sb = consts.tile([128, 1], f32, tag="bias", name="bias")
        zbias_sb = consts.tile([128, 1], f32, tag="zbias", name="zbias")
        bias0_sb = consts.tile([128, 1], f32, tag="bias0", name="bias0")
        ident = consts.tile([128, 128], f32, tag="ident", name="ident")
        zer_sb = consts.tile([128, NB], f32, tag="zer", name="zer")
        for cb in range(NBLK):
            nc.sync.dma_start(
                Wsb[:, 128 * cb : 128 * cb + 128], W_d[:, 128 * cb : 128 * cb + 128]
            )
        nc.sync.dma_start(bias_sb[:], bias_d[:, :])
        nc.sync.dma_start(zbias_sb[:], zbias_d[:, :])
        nc.sync.dma_start(bias0_sb[:], bias0_d[:, :])
        nc.sync.dma_start(ident[:], id_d[:, :])
        nc.sync.dma_start(zer_sb[:], zer_d[:, :])

        def wblk(cb):
            return Wsb[:, 128 * cb : 128 * cb + 128]

        Wbig = wblk(0)
        Xvar = [wblk(1 + j) for j in range(TCH)]
        Zvar = [wblk(17 + j) for j in range(TCH)]

        # --- X upfront load: 8 tiles of [128, 2048], emitted chunk-major ---
        xtiles = [
            xpool.tile([128, T * I], f32, tag=f"x{i}", name=f"x{i}")
            for i in range(8)
        ]
        ncc = (t_steps * I + 63) // 64
        for cc in range(ncc):
            c0, c1 = cc * 64, min(cc * 64 + 64, T * I)
            for i in range(8):
                nc.sync.dma_start(
                    xtiles[i][:, c0:c1], X_d[i * 128 : i * 128 + 128, c0:c1]
                )

        # --- initial state: zeros (via DMA from host zeros; memset is unsafe) ---
        s_cur = {}
        for c in range(NCH):
            s0 = spool.tile([128, NB], f32, tag=f"s{c}", name=f"s{c}")
            nc.sync.dma_start(s0[:], zer_d[:, :])
            s_cur[c] = s0

        # --- x staging: XT [128, 256]: row 4*dt+i = x_i(t0+dt) lane A (dt<16),
        #     row 64+4*dt+i = lane B. Staged per 16-t chunk via 4 PE transposes
        #     + 2 DVE copies + 1 partition-shift DMA.
        xt_cur = {}
        xt_next = {}
        px_cur = {}
        xb_cur = {}

        def stage_op(c, n16, r):
            cols = slice(64 * n16, 64 * n16 + 64)
            if r == 0:
                px_cur[c] = pxpool.tile(
                    [128, 512], f32, tag=f"px{c}", name=f"px{c}"
                )
                nc.tensor.transpose(
                    px_cur[c][0:64, 0:128], xtiles[4 * c][:, cols], ident[:]
                )
            elif r == 1:
                nc.tensor.transpose(
                    px_cur[c][0:64, 128:256], xtiles[4 * c + 1][:, cols], ident[:]
                )
            elif r == 2:
                xt_next[c] = xtpool.tile(
                    [128, NB], f32, tag=f"xt{c}", name=f"xt{c}"
                )
                nc.vector.tensor_copy(xt_next[c][0:64, :], px_cur[c][0:64, 0:256])
            elif r == 3:
                nc.tensor.transpose(
                    px_cur[c][0:64, 256:384], xtiles[4 * c + 2][:, cols], ident[:]
                )
            elif r == 4:
                nc.tensor.transpose(
                    px_cur[c][0:64, 384:512], xtiles[4 * c + 3][:, cols], ident[:]
                )
            elif r == 5:
                xb_cur[c] = xbpool.tile([64, NB], f32, tag=f"xb{c}", name=f"xb{c}")
                nc.vector.tensor_copy(xb_cur[c][:], px_cur[c][0:64, 256:512])
            elif r == 6:
                nc.sync.dma_start(xt_next[c][64:128, :], xb_cur[c][:])

        for c in range(NCH):
            for r in range(7):
                stage_op(c, 0, r)
            xt_cur[c] = xt_next[c]

        pz_cur = {}

        # --- main loop ---
        # Round t (0..T): computes h0_t (rows 0-31/64-95, skipped at t=T) and
        # h1_{t-1} (rows 32-63/96-127). z output index q = t-1.
        for t in range(t_steps + 1):
            last = t == t_steps
            j16 = t % TCH
            k16 = t // TCH
            for c in range(NCH):
                s = s_cur[c]
                ps = pspool.tile([128, 512], f32, tag=f"ps{c}", name=f"ps{c}")
                mm = nc.tensor.matmul
                if not last:
                    mm(ps[:, 0:NB], Wbig, s[:, :], start=True, stop=False,
                       skip_group_check=True)
                    mm(ps[:, 0:NB], Xvar[j16], xt_cur[c][:, :], start=False,
                       stop=True, skip_group_check=True)
                else:
                    mm(ps[:, 0:NB], Wbig, s[:, :], start=True, stop=True,
                       skip_group_check=True)

                s_new = spool.tile([128, NB], f32, tag=f"s{c}", name=f"s{c}")
                # t=0: s_init=0 makes the h1 rows of ps zero; bias0 has zeroed
                # b1 slots so tanh(0+0)=0 = h1_{-1}. t=T: h0 rows are unused
                # (z weights there are 0) but still defined.
                nc.scalar.activation(s_new[:], ps[:, 0:NB], AF.Tanh,
                                     bias=(bias0_sb[:] if t == 0 else bias_sb[:]))
                s_cur[c] = s_new

                if t > 0:
                    # head: z_q = W_ll @ h1_q + b_ll with q = t-1
                    q = t - 1
                    jq = q % TCH
                    kq = q // TCH
                    if jq == 0:
                        pz_cur[c] = pzpool.tile([128, 512], f32, tag=f"pz{c}",
                                                name=f"pz{c}")
                    pz = pz_cur[c]
                    mm(pz[:, 0:NB], Zvar[jq], s_new[:, :], start=(jq == 0),
                       stop=(jq == TCH - 1), skip_group_check=True)

                    if jq == TCH - 1 or last:
                        zs = zspool.tile([128, NB], f32, tag=f"zs{c}",
                                         name=f"zs{c}")
                        nc.vector.tensor_scalar_add(zs[:], pz[:, 0:NB],
                                                    zbias_sb[:])
                        ncols = 4 * (q - TCH * kq + 1)
                        for half in range(2):
                            pt = ptpool.tile([128, 512], f32, tag="pt", name="pt")
                            nc.tensor.transpose(
                                pt[:, 0:128],
                                zs[:, 128 * half : 128 * half + 128], ident[:],
                            )
                            ptsb = otpool.tile([128, 128], f32, tag="ot",
                                               name="ot")
                            nc.vector.tensor_copy(ptsb[:], pt[:, 0:128])
                            ob = 64 * kq
                            rB = BS // 2 * c + 256 + 128 * half
                            rA = BS // 2 * c + 128 * half
                            nc.sync.dma_start(
                                out_d[rB : rB + 128, ob : ob + ncols],
                                ptsb[:, 0:ncols],
                            )
                            nc.sync.dma_start(
                                out_d[rA : rA + 128, ob : ob + ncols],
                                ptsb[:, 64 : 64 + ncols],
                            )

                # spread staging of x-chunk k16+1 across this chunk's rounds
                if not last:
                    n16 = k16 + 1
                    if j16 <= 6 and n16 * TCH < t_steps:
                        stage_op(c, n16, j16)
            if not last and j16 == TCH - 1:
                for c in range(NCH):
                    if (k16 + 1) * TCH < t_steps:
                        xt_cur[c] = xt_next[c]

    nc.compile()
    return nc


def _pack_consts(W_ih0, W_hh0, b_ih0, b_hh0, W_ih1, W_hh1, b_ih1, b_hh1, W_ll, b_ll):
    f = np.float32
    Wnp = np.zeros((128, 128 * NBLK), f)
    # Wbig (block 0): ps = Wbig.T @ s
    WB = Wnp[:, 0:128]
    for base in (0, 64):
        WB[base : base + 32, base : base + 32] = W_hh0.T
        WB[base : base + 32, base + 32 : base + 64] = W_ih1.T
        WB[base + 32 : base + 64, base + 32 : base + 64] = W_hh1.T
    for j in range(TCH):
        XV = Wnp[:, 128 * (1 + j) : 128 * (2 + j)]
        XV[4 * j : 4 * j + 4, 0:32] = W_ih0.T
        XV[64 + 4 * j : 64 + 4 * j + 4, 64:96] = W_ih0.T
        ZV = Wnp[:, 128 * (17 + j) : 128 * (18 + j)]
        ZV[32:64, 64 + 4 * j : 64 + 4 * j + 4] = W_ll.T
        ZV[96:128, 4 * j : 4 * j + 4] = W_ll.T
    b0 = (b_ih0 + b_hh0).astype(f)
    b1 = (b_ih1 + b_hh1).astype(f)
    bias128 = np.concatenate([b0, b1, b0, b1]).reshape(128, 1).astype(f)
    z32 = np.zeros(32, f)
    bias0 = np.concatenate([b0, z32, b0, z32]).reshape(128, 1).astype(f)
    zbias = np.tile(b_ll.astype(f), 32).reshape(128, 1)
    ident = np.eye(128, dtype=f)
    zer = np.zeros((128, NB), f)
    return Wnp, bias128, zbias, ident, zer, bias0


def kernel(X, W_ih0, W_hh0, b_ih0, b_hh0, W_ih1, W_hh1, b_ih1, b_hh1, W_ll, b_ll,
           _collect=None):
    from concourse import bass_utils

    t_steps = T
    if "nc" not in _CACHE:
        _CACHE["nc"] = _build(t_steps)
    nc = _CACHE["nc"]

    Wnp, bias128, zbias, ident, zer, bias0 = _pack_consts(
        np.asarray(W_ih0), np.asarray(W_hh0), np.asarray(b_ih0), np.asarray(b_hh0),
        np.asarray(W_ih1), np.asarray(W_hh1), np.asarray(b_ih1), np.asarray(b_hh1),
        np.asarray(W_ll), np.asarray(b_ll),
    )
    Xf = np.ascontiguousarray(np.asarray(X), dtype=np.float32).reshape(B, T * I)
    in_maps = []
    for c in range(NCORES):
        in_maps.append({
            "Xs": Xf[c * BS : (c + 1) * BS],
            "Wstk": Wnp, "bias128": bias128, "zbias": zbias, "ident": ident,
            "zer": zer, "bias0": bias0,
        })

    kwargs = {}
    if _collect is not None:
        kwargs = {k: v for k, v in _collect.items() if k != "res"}
    res = bass_utils.run_bass_kernel_spmd(
        nc, in_maps, core_ids=list(range(NCORES)), **kwargs
    )
    out = np.concatenate([r["out"] for r in res.results], axis=0)
    if _collect is not None:
        _collect["res"] = res
    return out


if __name__ == "__main__":
    nc = _build(int(os.environ.get("RNN_T", "32")))
    print("built ok")


# revision 11
# speedup vs baseline: 1.8706x; 1.8706x over previous
"""Trainium2 Bass kernel for a 2-layer tanh RNN (batch_first) + Linear head.

Problem: X[8192, 512, 4] -> out[8192, 2048] with H=32 hidden units per layer.
Sharding: pure data parallelism over batch across 8 NeuronCores (1024 b/core).

Per-core design (ACT/tanh-bound):
  - State s_t = [h0_t; h1_{t-1}] per lane (the layer-1 update is skewed one
    step so both layers update from available inputs in ONE matmul round +
    ONE tanh). Two lanes of 256 batch share each [128, 256] state tile.
  - 2 independent "chains" (2 lanes x 256 batch each) skewed in time so the
    scalar engine (tanh) never idles; PE work runs in the tanh shadow.
  - Each chain-step: 3 full [128,128]x[128,256] matmuls into one PSUM bank:
      1. Wbig (block-structured W_hh0/W_ih1/W_hh1 for both lanes)  @ s_{t-1}
      2. Xvar_{t%16} (row-selector x-injection W_ih0)              @ XT tile
      3. Zvar_{t%16} (head W_ll with per-t output-row placement)   @ s_t
    then one activation(Tanh, bias) [128, 256] PSUM->SBUF for the new state.
  - The head accumulates 16 timesteps into one PSUM bank; every 16 steps it
    is copied (+b_ll) to SBUF, PE-transposed back to [batch, (t,o)] layout,
    copied to SBUF and DMA'd to HBM.
  - X is loaded up-front into SBUF ([128b, 2048] tiles) and PE-transposed
    into [4i x 16t rows, batch] staging tiles, spread across rounds so the
    transposes hide in the tanh shadow.
"""

import os
import sys
import numpy as np

for _p in ("/opt/trn_rl_repo",):
    if _p not in sys.path:
        sys.path.insert(0, _p)

B, T, I, H, O = 8192, 512, 4, 32, 4
NCORES = 8
BS = B // NCORES          # 1024 batch per core
NB = 256                  # batch columns per chain (2 lanes of 256 -> 512 b)
NCH = 2                   # chains per core
TCH = 16                  # timesteps per x staging chunk and per z PSUM bank
NBLK = 33                 # weight blocks: Wbig + 16 Xvar + 16 Zvar

_CACHE = {}


def _build(t_steps):
    import concourse.bass as bass  # noqa: F401
    import concourse.tile as tile
    from concourse import bacc, mybir
    from contextlib import ExitStack

    f32 = mybir.dt.float32
    f32r = mybir.dt.float32r
    AF = mybir.ActivationFunctionType

    nc = bacc.Bacc(
        "TRN2", target_bir_lowering=False, debug=False, num_devices=NCORES
    )

    X_d = nc.dram_tensor("Xs", [BS, T * I], f32, kind="ExternalInput").ap()
    W_d = nc.dram_tensor("Wstk", [128, 128 * NBLK], f32, kind="ExternalInput").ap()
    bias_d = nc.dram_tensor("bias128", [128, 1], f32, kind="ExternalInput").ap()
    zbias_d = nc.dram_tensor("zbias", [128, 1], f32, kind="ExternalInput").ap()
    bias0_d = nc.dram_tensor("bias0", [128, 1], f32, kind="ExternalInput").ap()
    id_d = nc.dram_tensor("ident", [128, 128], f32, kind="ExternalInput").ap()
    zer_d = nc.dram_tensor("zer", [128, NB], f32, kind="ExternalInput").ap()
    out_d = nc.dram_tensor("out", [BS, T * O], f32, kind="ExternalOutput").ap()

    with tile.TileContext(nc) as tc, ExitStack() as ctx:
        consts = ctx.enter_context(tc.tile_pool(name="consts", bufs=1))
        xpool = ctx.enter_context(tc.tile_pool(name="xdata", bufs=1))
        spool = ctx.enter_context(tc.tile_pool(name="state", bufs=2))
        xtpool = ctx.enter_context(tc.tile_pool(name="xstage", bufs=2))
        xbpool = ctx.enter_context(tc.tile_pool(name="xbtmp", bufs=2))
        zspool = ctx.enter_context(tc.tile_pool(name="zstage", bufs=2))
        otpool = ctx.enter_context(tc.tile_pool(name="otstage", bufs=4))
        pspool = ctx.enter_context(tc.tile_pool(name="ps", bufs=1, space="PSUM"))
        pzpool = ctx.enter_context(tc.tile_pool(name="pz", bufs=1, space="PSUM"))
        pxpool = ctx.enter_context(tc.tile_pool(name="px", bufs=1, space="PSUM"))
        ptpool = ctx.enter_context(tc.tile_pool(name="pt", bufs=2, space="PSUM"))

        # --- constants / weights ---
        Wsb = consts.tile([128, 128 * NBLK], f32r, tag="wstk", name="wstk")
        bias_sb = consts.tile([128, 1], f32, tag="bias", name="bias")
        zbias_sb = consts.tile([128, 1], f32, tag="zbias", name="zbias")
        bias0_sb = consts.tile([128, 1], f32, tag="bias0", name="bias0")
        ident = consts.tile([128, 128], f32, tag="ident", name="ident")
        zer_sb = consts.tile([128, NB], f32, tag="zer", name="zer")
        for cb in range(NBLK):
            nc.sync.dma_start(
                Wsb[:, 128 * cb : 128 * cb + 128],
                W_d[:, 128 * cb : 128 * cb + 128].bitcast(f32r),
            )
        nc.sync.dma_start(bias_sb[:], bias_d[:, :])
        nc.sync.dma_start(zbias_sb[:], zbias_d[:, :])
        nc.sync.dma_start(bias0_sb[:], bias0_d[:, :])
        nc.sync.dma_start(ident[:], id_d[:, :])
        nc.sync.dma_start(zer_sb[:], zer_d[:, :])

        def wblk(cb):
            return Wsb[:, 128 * cb : 128 * cb + 128]

        Wbig = wblk(0)
        Xvar = [wblk(1 + j) for j in range(TCH)]
        Zvar = [wblk(17 + j) for j in range(TCH)]

        # --- X upfront load: 8 tiles of [128, 2048], emitted chunk-major ---
        xtiles = [
            xpool.tile([128, T * I], f32, tag=f"x{i}", name=f"x{i}")
            for i in range(8)
        ]
        ncc = (t_steps * I + 63) // 64
        for cc in range(ncc):
            c0, c1 = cc * 64, min(cc * 64 + 64, T * I)
            for i in range(8):
                nc.sync.dma_start(
                    xtiles[i][:, c0:c1], X_d[i * 128 : i * 128 + 128, c0:c1]
                )

        # --- initial state: zeros (via DMA from host zeros; memset is unsafe) ---
        s_cur = {}
        for c in range(NCH):
            s0 = spool.tile([128, NB], f32r, tag=f"s{c}", name=f"s{c}")
            nc.sync.dma_start(s0[:], zer_d[:, :].bitcast(f32r))
            s_cur[c] = s0

        # --- x staging: XT [128, 256]: row 4*dt+i = x_i(t0+dt) lane A (dt<16),
        #     row 64+4*dt+i = lane B. Staged per 16-t chunk via 4 PE transposes
        #     + 2 DVE copies + 1 partition-shift DMA.
        xt_cur = {}
        xt_next = {}
        px_cur = {}
        xb_cur = {}

        def stage_op(c, n16, r):
            cols = slice(64 * n16, 64 * n16 + 64)
            if r == 0:
                px_cur[c] = pxpool.tile(
                    [128, 512], f32, tag=f"px{c}", name=f"px{c}"
                )
                nc.tensor.transpose(
                    px_cur[c][0:64, 0:128], xtiles[4 * c][:, cols], ident[:]
                )
            elif r == 1:
                nc.tensor.transpose(
                    px_cur[c][0:64, 128:256], xtiles[4 * c + 1][:, cols], ident[:]
                )
            elif r == 2:
                xt_next[c] = xtpool.tile(
                    [128, NB], f32r, tag=f"xt{c}", name=f"xt{c}"
                )
                nc.vector.tensor_copy(xt_next[c][0:64, :], px_cur[c][0:64, 0:256])
            elif r == 3:
                nc.tensor.transpose(
                    px_cur[c][0:64, 256:384], xtiles[4 * c + 2][:, cols], ident[:]
                )
            elif r == 4:
                nc.tensor.transpose(
                    px_cur[c][0:64, 384:512], xtiles[4 * c + 3][:, cols], ident[:]
                )
            elif r == 5:
                xb_cur[c] = xbpool.tile([64, NB], f32r, tag=f"xb{c}", name=f"xb{c}")
                nc.vector.tensor_copy(xb_cur[c][:], px_cur[c][0:64, 256:512])
            elif r == 6:
                nc.sync.dma_start(xt_next[c][64:128, :], xb_cur[c][:])

        for c in range(NCH):
            for r in range(7):
                stage_op(c, 0, r)
            xt_cur[c] = xt_next[c]

        pz_cur = {}

        # --- main loop ---
        # Round t (0..T): computes h0_t (rows 0-31/64-95, skipped at t=T) and
        # h1_{t-1} (rows 32-63/96-127). z output index q = t-1.
        for t in range(t_steps + 1):
            last = t == t_steps
            j16 = t % TCH
            k16 = t // TCH
            for c in range(NCH):
                s = s_cur[c]
                ps = pspool.tile([128, 512], f32, tag=f"ps{c}", name=f"ps{c}")
                mm = nc.tensor.matmul
                if not last:
                    mm(ps[:, 0:NB], Wbig, s[:, :], start=True, stop=False,
                       skip_group_check=True)
                    mm(ps[:, 0:NB], Xvar[j16], xt_cur[c][:, :], start=False,
                       stop=True, skip_group_check=True)
                else:
                    mm(ps[:, 0:NB], Wbig, s[:, :], start=True, stop=True,
                       skip_group_check=True)

                s_new = spool.tile([128, NB], f32r, tag=f"s{c}", name=f"s{c}")
                # t=0: s_init=0 makes the h1 rows of ps zero; bias0 has zeroed
                # b1 slots so tanh(0+0)=0 = h1_{-1}. t=T: h0 rows are unused
                # (z weights there are 0) but still defined.
                nc.scalar.activation(s_new[:], ps[:, 0:NB], AF.Tanh,
                                     bias=(bias0_sb[:] if t == 0 else bias_sb[:]))
                s_cur[c] = s_new

                if t > 0:
                    # head: z_q = W_ll @ h1_q + b_ll with q = t-1
                    q = t - 1
                    jq = q % TCH
                    kq = q // TCH
                    if jq == 0:
                        pz_cur[c] = pzpool.tile([128, 512], f32, tag=f"pz{c}",
                                                name=f"pz{c}")
                    pz = pz_cur[c]
                    mm(pz[:, 0:NB], Zvar[jq], s_new[:, :], start=(jq == 0),
                       stop=(jq == TCH - 1), skip_group_check=True)

                    if jq == TCH - 1 or last:
                        zs = zspool.tile([128, NB], f32, tag=f"zs{c}",
                                         name=f"zs{c}")
                        nc.vector.tensor_scalar_add(zs[:], pz[:, 0:NB],
                                                    zbias_sb[:])
                        ncols = 4 * (q - TCH * kq + 1)
                        for half in range(2):
                            pt = ptpool.tile([128, 512], f32, tag="pt", name="pt")
                            nc.tensor.transpose(
                                pt[:, 0:128],
                                zs[:, 128 * half : 128 * half + 128], ident[:],
                            )
                            ptsb = otpool.tile([128, 128], f32, tag="ot",
                                               name="ot")
                            nc.vector.tensor_copy(ptsb[:], pt[:, 0:128])
                            ob = 64 * kq
                            rB = BS // 2 * c + 256 + 128 * half
                            rA = BS // 2 * c + 128 * half
                            nc.sync.dma_start(
                                out_d[rB : rB + 128, ob : ob + ncols],
                                ptsb[:, 0:ncols],
                            )
                            nc.sync.dma_start(
                                out_d[rA : rA + 128, ob : ob + ncols],
                                ptsb[:, 64 : 64 + ncols],
                            )

                # spread staging of x-chunk k16+1 across this chunk's rounds
                if not last:
                    n16 = k16 + 1
                    if j16 <= 6 and n16 * TCH < t_steps:
                        stage_op(c, n16, j16)
            if not last and j16 == TCH - 1:
                for c in range(NCH):
                    if (k16 + 1) * TCH < t_steps:
                        xt_cur[c] = xt_next[c]

    nc.compile()
    return nc


def _pack_consts(W_ih0, W_hh0, b_ih0, b_hh0, W_ih1, W_hh1, b_ih1, b_hh1, W_ll, b_ll):
    f = np.float32
    Wnp = np.zeros((128, 128 * NBLK), f)
    # Wbig (block 0): ps = Wbig.T @ s
    WB = Wnp[:, 0:128]
    for base in (0, 64):
        WB[base : base + 32, base : base + 32] = W_hh0.T
        WB[base : base + 32, base + 32 : base + 64] = W_ih1.T
        WB[base + 32 : base + 64, base + 32 : base + 64] = W_hh1.T
    for j in range(TCH):
        XV = Wnp[:, 128 * (1 + j) : 128 * (2 + j)]
        XV[4 * j : 4 * j + 4, 0:32] = W_ih0.T
        XV[64 + 4 * j : 64 + 4 * j + 4, 64:96] = W_ih0.T
        ZV = Wnp[:, 128 * (17 + j) : 128 * (18 + j)]
        ZV[32:64, 64 + 4 * j : 64 + 4 * j + 4] = W_ll.T
        ZV[96:128, 4 * j : 4 * j + 4] = W_ll.T
    b0 = (b_ih0 + b_hh0).astype(f)
    b1 = (b_ih1 + b_hh1).astype(f)
    bias128 = np.concatenate([b0, b1, b0, b1]).reshape(128, 1).astype(f)
    z32 = np.zeros(32, f)
    bias0 = np.concatenate([b0, z32, b0, z32]).reshape(128, 1).astype(f)
    zbias = np.tile(b_ll.astype(f), 32).reshape(128, 1)
    ident = np.eye(128, dtype=f)
    zer = np.zeros((128, NB), f)
    return Wnp, bias128, zbias, ident, zer, bias0


def kernel(X, W_ih0, W_hh0, b_ih0, b_hh0, W_ih1, W_hh1, b_ih1, b_hh1, W_ll, b_ll,
           _collect=None):
    from concourse import bass_utils

    t_steps = T
    if "nc" not in _CACHE:
        _CACHE["nc"] = _build(t_steps)
    nc = _CACHE["nc"]

    Wnp, bias128, zbias, ident, zer, bias0 = _pack_consts(
        np.asarray(W_ih0), np.asarray(W_hh0), np.asarray(b_ih0), np.asarray(b_hh0),
        np.asarray(W_ih1), np.asarray(W_hh1), np.asarray(b_ih1), np.asarray(b_hh1),
        np.asarray(W_ll), np.asarray(b_ll),
    )
    Xf = np.ascontiguousarray(np.asarray(X), dtype=np.float32).reshape(B, T * I)
    in_maps = []
    for c in range(NCORES):
        in_maps.append({
            "Xs": Xf[c * BS : (c + 1) * BS],
            "Wstk": Wnp, "bias128": bias128, "zbias": zbias, "ident": ident,
            "zer": zer, "bias0": bias0,
        })

    kwargs = {}
    if _collect is not None:
        kwargs = {k: v for k, v in _collect.items() if k != "res"}
    res = bass_utils.run_bass_kernel_spmd(
        nc, in_maps, core_ids=list(range(NCORES)), **kwargs
    )
    out = np.concatenate([r["out"] for r in res.results], axis=0)
    if _collect is not None:
        _collect["res"] = res
    return out


if __name__ == "__main__":
    nc = _build(int(os.environ.get("RNN_T", "32")))
    print("built ok")
